# revision 46
# baseline (speedup 1.0000x reference)
"""Trainium2 Bass kernel for the vq_codebook contrastive-loss module.

Strategy (data-parallel over batch axis B=8, one batch of P=784 anchors per core):
  - Each core: l2-normalize its feat slice + the full codebook, computes
    dist = nf @ ncbk.T in full fp32 (argmax must match the fp32 reference),
    takes per-row argmax -> ind_c [784].
  - AllGather(ind) across the 8 cores -> ind_all [6272] (global anchor order).
  - norm_vq rows are normalized-codebook rows (l2norm(codebook[ind]) ==
    l2norm(codebook)[ind]).
  - In-batch codebook distances via factorization: R = nvq_own @ ncbk.T
    [784, 512] once (fp32r), then cbk_dist[p, q] = R[p, ind_q] is a gpsimd
    ap_gather along the free axis -- no per-q-tile matmul/gather/transpose.
  - All similarity logits are dot products of unit vectors, so
    |cs/TEMP| <= 1/0.07 and exp() cannot overflow; the reference's row-max
    subtraction cancels exactly:
        num = -sum_pos(cs)/T + cnt*log(sum exp(cs/T)*orf)
    -> fully streaming single pass, no materialized [P, BP] blocks.
  - Self-exclusion of the in-batch positive mask is data-driven: a
    per-core host-provided selfcol vector compared against a column iota
    (handles the core-dependent, non-128-aligned diagonal in SPMD).
  - Row normalization is folded into the PE transposes (matmul against
    diag(rsqrt(sumsq))); mask side-ops (iota-compare, or-mask, diag build)
    run on the otherwise-idle gpsimd engine.
  - Device outputs per core: [num_batch, cnt_batch, num_bank, cnt_bank].
    Host computes (sum(num_b/cnt_b) + sum(num_k/cnt_k)) / 16; the bank block
    has zero positives for this data so 0/0 -> NaN, matching the reference.
"""

import os
import sys

import numpy as np

sys.path.insert(0, "/opt/trn_rl_repo")

from concourse import bacc, bass, mybir, tile  # noqa: E402
from concourse.bass_utils import run_bass_kernel_spmd  # noqa: E402
from concourse.masks import make_identity  # noqa: E402

F32 = mybir.dt.float32
F32R = mybir.dt.float32r
U32 = mybir.dt.uint32
I16 = mybir.dt.int16
ALU = mybir.AluOpType
ACTF = mybir.ActivationFunctionType
AX = mybir.AxisListType

B, P, D, PD, NC, NB = 8, 784, 768, 128, 512, 8192
BP = B * P                      # 6272
PT = 7                          # anchor tiles of 128 (rows 784..895 are pad)
DK = D // 128                   # 6 contraction chunks
INV_TEMP = 1.0 / 0.07
POS_T, NEG_T = 0.5, 0.1
QTILES = [512] * 12 + [128]     # in-batch q tiling (6272)
JTILES = [512] * 16             # bank tiling (8192)
NCORES = 8


class K:
    """Kernel builder state."""

    def __init__(self, nc, tc, io, pools):
        self.nc = nc
        self.tc = tc
        self.io = io
        self.pools = pools
        self.eye = None
        self.padmask = None
        self.selfsb = None
        self.qiota = None

    def norm_diag(self, src_tile, want_r=False):
        """Row sumsq -> rsqrt -> 128x128 diag(rsqrt) for a [128, D'] tile."""
        nc, pools = self.nc, self.pools
        ss = pools["small"].tile([128, 1], F32, tag="ss", bufs=3)
        junk = pools["junk"].tile(
            [128, src_tile.shape[1]], F32, tag="nrm_junk", bufs=2
        )
        nc.scalar.activation(out=junk, in_=src_tile, func=ACTF.Square, accum_out=ss)
        sm = pools["small"].tile([128, 1], F32, tag="sm", bufs=3)
        nc.vector.tensor_scalar_max(out=sm, in0=ss, scalar1=1e-12)
        rec = pools["small"].tile([128, 1], F32, tag="rec", bufs=3)
        nc.vector.reciprocal(out=rec, in_=sm)
        r = pools["small"].tile([128, 1], F32, tag="rnorm", bufs=3)
        nc.scalar.activation(out=r, in_=rec, func=ACTF.Sqrt)
        diagr = pools["diag"].tile([128, 128], F32, tag="diagr", bufs=3)
        nc.vector.tensor_scalar_mul(out=diagr, in0=self.eye, scalar1=r)
        if want_r:
            return diagr, r
        return diagr

    def transpose_group(self, srcs, idents, dst_chunks, col0, nkc, scaled):
        """Transpose a group of up to 4 [128, nkc*128] tiles into
        dst_chunks[k][:, col0 : col0+len(srcs)*128] with one batched
        PSUM->SBUF copy per k-chunk.

        scaled: idents are diag(rsqrt) -> matmul; else true transpose.
        """
        nc, pools = self.nc, self.pools
        grouped = False
        g = len(srcs)
        for k in range(nkc):
            if grouped:
                pt = pools["psT"].tile([128, 512], F32, tag="ptT", bufs=2,
                                       space="PSUM")
                for s, (src_tile, ident) in enumerate(zip(srcs, idents)):
                    src = src_tile[:, k * 128:(k + 1) * 128]
                    dst = pt[:, s * 128:(s + 1) * 128]
                    if scaled:
                        nc.tensor.matmul(out=dst, lhsT=src, rhs=ident,
                                         start=True, stop=True)
                    else:
                        nc.tensor.transpose(out=dst, in_=src, identity=ident)
                nc.any.tensor_copy(out=dst_chunks[k][:, col0:col0 + g * 128],
                                   in_=pt[:, :g * 128])
            else:
                for s, (src_tile, ident) in enumerate(zip(srcs, idents)):
                    pt = pools["psT"].tile([128, 128], F32, tag="ptT", bufs=2,
                                           space="PSUM")
                    src = src_tile[:, k * 128:(k + 1) * 128]
                    if scaled:
                        nc.tensor.matmul(out=pt, lhsT=src, rhs=ident,
                                         start=True, stop=True)
                    else:
                        nc.tensor.transpose(out=pt, in_=src, identity=ident)
                    nc.any.tensor_copy(
                        out=dst_chunks[k][:, col0 + s * 128:col0 + (s + 1) * 128],
                        in_=pt,
                    )

    def load_norm_transpose(self, dram, nrows, ncols, dst_chunks, ld_tag,
                            save_norm_to=None):
        """Load [nrows, ncols] row-major, l2-normalize rows, transpose into
        dst_chunks (ncols//128 chunks of [128, nrows_padded])."""
        nc, pools = self.nc, self.pools
        nkc = ncols // 128
        nt = (nrows + 127) // 128
        for g0 in range(0, nt, 1):
            gts = []
            idents = []
            for i in range(g0, min(g0 + 1, nt)):
                r0, r1 = i * 128, min((i + 1) * 128, nrows)
                rr = r1 - r0
                raw = pools["ld"].tile([128, ncols], F32, tag=ld_tag, bufs=3)
                if rr < 128:
                    nc.vector.memset(raw, 0.0)
                nc.sync.dma_start(out=raw[:rr, :], in_=dram[r0:r1, :])
                diagr, r = self.norm_diag(raw, want_r=True)
                if save_norm_to is not None:
                    nrm = pools["ld"].tile([128, ncols], F32, tag=ld_tag + "n",
                                           bufs=1)
                    nc.scalar.activation(out=nrm, in_=raw, func=ACTF.Copy,
                                         scale=r)
                    nc.sync.dma_start(out=save_norm_to[r0:r1, :],
                                      in_=nrm[:rr, :])
                gts.append(raw)
                idents.append(diagr)
            self.transpose_group(gts, idents, dst_chunks, g0 * 128, nkc,
                                 scaled=True)

    def contrast_tile(self, ti, w, p_cs, cbk, cbk_is_psum, cntp, t2p, s1p,
                      self_q0=None, self_m=None):
        """One [128, w] tile of the streaming NCE accumulation.

        cbk: codebook-distance tile (PSUM for bank, SBUF for in-batch).
        Accumulates partials into column ti of cntp/t2p/s1p. If self_q0 is
        set (in-batch), the positive mask excludes each row's own global
        column: posf = (cbk > .5) * (qiota + q0 != selfcol[row]).
        """
        nc, pools = self.nc, self.pools
        sc = pools["scr"]
        posf = sc.tile([128, 512], F32, tag="posf", bufs=2)
        if self_q0 is not None:
            neq = sc.tile([128, 512], F32, tag="neq", bufs=2)
            nc.vector.tensor_scalar(
                out=neq[:, :w], in0=self.qiota[:, :w],
                scalar1=float(self_q0),
                scalar2=self.selfsb[:, self_m:self_m + 1],
                op0=ALU.add, op1=ALU.not_equal,
            )
            nc.vector.scalar_tensor_tensor(
                out=posf[:, :w], in0=cbk[:, :w], scalar=POS_T,
                in1=neq[:, :w], op0=ALU.is_gt, op1=ALU.mult,
                accum_out=cntp[:, ti:ti + 1],
            )
        else:
            nc.vector.tensor_scalar(
                out=posf[:, :w], in0=cbk[:, :w], scalar1=POS_T, scalar2=None,
                op0=ALU.is_gt, op1=ALU.add, accum_out=cntp[:, ti:ti + 1],
            )
        junk = pools["junk"].tile([128, 512], F32, tag="sjunk1", bufs=2)
        nc.vector.scalar_tensor_tensor(
            out=junk[:, :w], in0=p_cs[:, :w], scalar=1.0, in1=posf[:, :w],
            op0=ALU.mult, op1=ALU.mult, accum_out=t2p[:, ti:ti + 1],
        )
        orf = sc.tile([128, 512], F32, tag="orf", bufs=2)
        orf_engine = nc.vector if cbk_is_psum else nc.gpsimd
        orf_engine.scalar_tensor_tensor(
            out=orf[:, :w], in0=cbk[:, :w], scalar=NEG_T, in1=posf[:, :w],
            op0=ALU.is_lt, op1=ALU.add,
        )
        e_t = sc.tile([128, 512], F32, tag="e_t", bufs=2)
        nc.scalar.activation(
            out=e_t[:, :w], in_=p_cs[:, :w], func=ACTF.Exp, scale=INV_TEMP
        )
        junk2 = pools["junk"].tile([128, 512], F32, tag="sjunk2", bufs=2)
        nc.vector.scalar_tensor_tensor(
            out=junk2[:, :w], in0=e_t[:, :w], scalar=1.0, in1=orf[:, :w],
            op0=ALU.mult, op1=ALU.mult, accum_out=s1p[:, ti:ti + 1],
        )

    def epilogue(self, m, parts_m, nt, numcol, cntcol):
        """Reduce partials of anchor-tile m into num/cnt columns."""
        nc, small = self.nc, self.pools["small"]
        cntp, t2p, s1p = parts_m
        cnt_m = small.tile([128, 1], F32, tag="cnt_m", bufs=2)
        t2_m = small.tile([128, 1], F32, tag="t2_m", bufs=2)
        s1_m = small.tile([128, 1], F32, tag="s1_m", bufs=2)
        nc.vector.tensor_reduce(out=cnt_m, in_=cntp[:, :nt], axis=AX.X, op=ALU.add)
        nc.vector.tensor_reduce(out=t2_m, in_=t2p[:, :nt], axis=AX.X, op=ALU.add)
        nc.vector.tensor_reduce(out=s1_m, in_=s1p[:, :nt], axis=AX.X, op=ALU.add)
        logden = small.tile([128, 1], F32, tag="logden", bufs=2)
        nc.scalar.activation(out=logden, in_=s1_m, func=ACTF.Ln)
        t2s = small.tile([128, 1], F32, tag="t2s", bufs=2)
        nc.vector.tensor_scalar_mul(out=t2s, in0=t2_m, scalar1=INV_TEMP)
        num_m = small.tile([128, 1], F32, tag="num_m", bufs=2)
        nc.vector.scalar_tensor_tensor(
            out=num_m, in0=cnt_m, scalar=logden, in1=t2s,
            op0=ALU.mult, op1=ALU.subtract,
        )
        if m == PT - 1:  # zero pad anchors (start partition must be 0/32/64/96)
            nc.vector.tensor_tensor(out=num_m, in0=num_m, in1=self.padmask,
                                    op=ALU.mult)
            nc.vector.tensor_tensor(out=cnt_m, in0=cnt_m, in1=self.padmask,
                                    op=ALU.mult)
        nc.vector.tensor_copy(out=numcol[:, m:m + 1], in_=num_m)
        nc.vector.tensor_copy(out=cntcol[:, m:m + 1], in_=cnt_m)


def build_kernel_body(nc, tc, io, pools):
    st = K(nc, tc, io, pools)
    sb = pools["sb"]
    small = pools["small"]
    psum = pools["psum"]

    eye = sb.tile([128, 128], F32, tag="eye")
    make_identity(nc, eye)
    st.eye = eye
    padmask = sb.tile([128, 1], F32, tag="padmask")
    nc.vector.memset(padmask, 0.0)
    nc.vector.memset(padmask[0:P - (PT - 1) * 128, :], 1.0)
    st.padmask = padmask
    selfsb = sb.tile([128, PT], F32, tag="selfsb")
    nc.sync.dma_start(
        out=selfsb, in_=io["selfcol"][:].rearrange("(m p) -> p m", p=128)
    )
    st.selfsb = selfsb
    qiota = sb.tile([128, 512], F32, tag="qiota")
    nc.gpsimd.iota(qiota, pattern=[[1, 512]], base=0, channel_multiplier=0,
                   allow_small_or_imprecise_dtypes=True)
    st.qiota = qiota

    # ---------- codebook: normalize rows, store to DRAM, transpose ----------
    ncbkT = [sb.tile([128, NC], F32, tag=f"ncbkT{k}", name=f"ncbkT{k}")
             for k in range(DK)]
    st.load_norm_transpose(io["cbk"], NC, D, ncbkT, "ld768",
                           save_norm_to=io["ncbk_dram"])
    # ---------- feat slice: normalize+transpose -> nfT [768, 896] ----------
    nfT = [sb.tile([128, PT * 128], F32, tag=f"nfT{k}", bufs=1, name=f"nfT{k}")
           for k in range(DK)]
    st.load_norm_transpose(io["feat"], P, D, nfT, "ld768")

    # ---------- dist (fp32, exact-ish) + argmax ----------
    ind_tiles = []
    for m in range(PT):
        pd = psum.tile([128, NC], F32, tag="pcbk", bufs=2, space="PSUM")
        for k in range(DK):
            nc.tensor.matmul(
                out=pd,
                lhsT=nfT[k][:, m * 128:(m + 1) * 128],
                rhs=ncbkT[k],
                start=(k == 0),
                stop=(k == DK - 1),
            )
        dist_sb = pools["ld"].tile([128, NC], F32, tag="dist_sb", bufs=1)
        nc.any.tensor_copy(out=dist_sb, in_=pd)
        mx8 = small.tile([128, 8], F32, tag="mx8", bufs=2)
        ix8 = small.tile([128, 8], U32, tag="ix8", bufs=PT)
        nc.vector.max(mx8, dist_sb)
        nc.vector.max_index(ix8, mx8, dist_sb)
        ind_tiles.append(ix8)
        rr = min(P, (m + 1) * 128) - m * 128
        nc.sync.dma_start(out=io["cc_in"][m * 128:m * 128 + rr], in_=ix8[:rr, 0:1])
        nc.sync.dma_start(
            out=io["ind_out"][m * 128:m * 128 + rr], in_=ix8[:rr, 0:1]
        )

    # ---------- proj/ema: normalize+transpose (independent of collective) --
    nprojT = sb.tile([128, PT * 128], F32R, tag="nprojT")
    st.load_norm_transpose(io["proj"], P, PD, [nprojT], "ld128")
    nemaT = sb.tile([128, BP], F32R, tag="nemaT", name="nemaT")
    st.load_norm_transpose(io["ema"], BP, PD, [nemaT], "ld128")
    nbemaT = sb.tile([128, NB], F32R, tag="nbemaT", name="nbemaT")
    st.load_norm_transpose(io["bema"], NB, PD, [nbemaT], "ld128")

    # ---------- AllGather indices ----------
    if os.environ.get("KNOCC"):
        # timeline-sim variant: no collectives supported; timing-equivalent
        # DMA stand-in (payload is 3KB either way)
        for c in range(NCORES):
            nc.sync.dma_start(out=io["cc_out"][c * P:(c + 1) * P],
                              in_=io["cc_in"][:])
    else:
        nc.gpsimd.collective_compute(
            "AllGather",
            ALU.bypass,
            ins=[io["cc_in"][:]],
            outs=[io["cc_out"][:]],
            replica_groups=[list(range(NCORES))],
        )

    # ---------- nvq (own anchors) = gather(ncbk, ind), transpose ----------
    nvqT = [sb.tile([128, PT * 128], F32R, tag=f"nvqT{k}", name=f"nvqT{k}")
            for k in range(DK)]
    gts = []
    for m in range(PT):
        rr = min(P, (m + 1) * 128) - m * 128
        g = pools["ld"].tile([128, D], F32, tag="ld768", bufs=3)
        if rr < 128:
            nc.vector.memset(g, 0.0)
        nc.gpsimd.indirect_dma_start(
            out=g[:rr, :],
            out_offset=None,
            in_=io["ncbk_dram"][:],
            in_offset=bass.IndirectOffsetOnAxis(ap=ind_tiles[m][:rr, 0:1], axis=0),
        )
        gts.append(g)
        if True:
            g0 = m + 1 - len(gts)
            st.transpose_group(gts, [eye] * len(gts), nvqT, g0 * 128, DK,
                               scaled=False)
            gts = []

    # ---------- R = nvq @ ncbk.T [896, 512] (fp32r) ----------
    R_tiles = []
    for m in range(PT):
        pr = psum.tile([128, NC], F32, tag="pcbk", bufs=2, space="PSUM")
        for k in range(DK):
            nc.tensor.matmul(
                out=pr,
                lhsT=nvqT[k][:, m * 128:(m + 1) * 128].bitcast(F32),
                rhs=ncbkT[k],
                start=(k == 0), stop=(k == DK - 1),
            )
        rt = sb.tile([128, NC], F32, tag=f"R{m}", name=f"R{m}")
        nc.any.tensor_copy(out=rt, in_=pr)
        R_tiles.append(rt)

    # ---------- in-batch contrastive ----------
    numcolB = sb.tile([128, PT], F32, tag="numcolB")
    cntcolB = sb.tile([128, PT], F32, tag="cntcolB")
    nqt = len(QTILES)
    partsB = {
        m: (
            pools["part"].tile([128, nqt], F32, tag=f"cntpB{m}", name=f"cntpB{m}"),
            pools["part"].tile([128, nqt], F32, tag=f"t2pB{m}", name=f"t2pB{m}"),
            pools["part"].tile([128, nqt], F32, tag=f"s1pB{m}", name=f"s1pB{m}"),
        )
        for m in range(PT)
    }
    for ti, w in enumerate(QTILES):
        q0 = ti * 512
        rhs = [pools["rhs"].tile([128, 512], F32R, tag=f"rhs{k}", bufs=2,
                                 name=f"rhsq{k}")
               for k in range(DK)]
        for s in range(w // 128):
            idx = small.tile([128, 1], U32, tag="qidx", bufs=4)
            nc.sync.dma_start(
                out=idx,
                in_=io["cc_out"][q0 + s * 128:q0 + (s + 1) * 128].unsqueeze(-1),
            )
            g = pools["ld"].tile([128, D], F32, tag="ld768", bufs=3)
            nc.gpsimd.indirect_dma_start(
                out=g,
                out_offset=None,
                in_=io["ncbk_dram"][:],
                in_offset=bass.IndirectOffsetOnAxis(ap=idx, axis=0),
            )
            st.transpose_group([g], [st.eye], rhs, s * 128, DK, scaled=False)
        for m in range(PT):
            cntp, t2p, s1p = partsB[m]
            p_cs = psum.tile([128, 512], F32, tag="pcs", bufs=2, space="PSUM")
            nc.tensor.matmul(
                out=p_cs[:, :w],
                lhsT=nprojT[:, m * 128:(m + 1) * 128],
                rhs=nemaT[:, q0:q0 + w],
                start=True, stop=True,
            )
            p_cbk = psum.tile([128, 512], F32, tag="pcbk", bufs=2, space="PSUM")
            for k in range(DK):
                nc.tensor.matmul(
                    out=p_cbk[:, :w],
                    lhsT=nvqT[k][:, m * 128:(m + 1) * 128],
                    rhs=rhs[k][:, :w],
                    start=(k == 0), stop=(k == DK - 1),
                )
            st.contrast_tile(ti, w, p_cs, p_cbk, True, cntp, t2p, s1p,
                             self_q0=q0, self_m=m)
    for m in range(PT):
        st.epilogue(m, partsB[m], nqt, numcolB, cntcolB)

    # ---------- bank contrastive ----------
    numcolK = sb.tile([128, PT], F32, tag="numcolK")
    cntcolK = sb.tile([128, PT], F32, tag="cntcolK")
    njt = len(JTILES)
    partsK = {
        m: (
            pools["part"].tile([128, njt], F32, tag=f"cntpK{m}", name=f"cntpK{m}"),
            pools["part"].tile([128, njt], F32, tag=f"t2pK{m}", name=f"t2pK{m}"),
            pools["part"].tile([128, njt], F32, tag=f"s1pK{m}", name=f"s1pK{m}"),
        )
        for m in range(PT)
    }
    for ti, w in enumerate(JTILES):
        j0 = ti * 512
        rhs = [pools["rhs"].tile([128, 512], F32R, tag=f"rhs{k}", bufs=2,
                                 name=f"rhs{k}")
               for k in range(DK)]
        for s in range(w // 128):
            raw = pools["ld"].tile([128, D], F32, tag="ld768", bufs=3)
            nc.sync.dma_start(
                out=raw, in_=io["bank"][j0 + s * 128:j0 + (s + 1) * 128, :]
            )
            st.transpose_group([raw], [st.norm_diag(raw)], rhs, s * 128, DK,
                               scaled=True)
        for m in range(PT):
            cntp, t2p, s1p = partsK[m]
            p_cs = psum.tile([128, 512], F32, tag="pcs", bufs=2, space="PSUM")
            nc.tensor.matmul(
                out=p_cs[:, :w],
                lhsT=nprojT[:, m * 128:(m + 1) * 128],
                rhs=nbemaT[:, j0:j0 + w],
                start=True, stop=True,
            )
            p_cbk = psum.tile([128, 512], F32, tag="pcbk", bufs=2, space="PSUM")
            for k in range(DK):
                nc.tensor.matmul(
                    out=p_cbk[:, :w],
                    lhsT=nvqT[k][:, m * 128:(m + 1) * 128],
                    rhs=rhs[k][:, :w],
                    start=(k == 0), stop=(k == DK - 1),
                )
            st.contrast_tile(ti, w, p_cs, p_cbk, True, cntp, t2p, s1p)
    for m in range(PT):
        st.epilogue(m, partsK[m], njt, numcolK, cntcolK)

    # ---------- final partition reduction -> out4 ----------
    vec4 = sb.tile([128, 4], F32, tag="vec4")
    nc.vector.tensor_reduce(out=vec4[:, 0:1], in_=numcolB, axis=AX.X, op=ALU.add)
    nc.vector.tensor_reduce(out=vec4[:, 1:2], in_=cntcolB, axis=AX.X, op=ALU.add)
    nc.vector.tensor_reduce(out=vec4[:, 2:3], in_=numcolK, axis=AX.X, op=ALU.add)
    nc.vector.tensor_reduce(out=vec4[:, 3:4], in_=cntcolK, axis=AX.X, op=ALU.add)
    ones = sb.tile([128, 1], F32, tag="ones")
    nc.vector.memset(ones, 1.0)
    p_out = psum.tile([1, 4], F32, tag="pout", bufs=1, space="PSUM")
    nc.tensor.matmul(out=p_out, lhsT=ones, rhs=vec4, start=True, stop=True)
    out_sb = sb.tile([1, 4], F32, tag="out_sb")
    nc.vector.tensor_copy(out=out_sb, in_=p_out)
    nc.sync.dma_start(out=io["out4"][:], in_=out_sb)


def build_nc():
    nc = bacc.Bacc(
        "TRN2", target_bir_lowering=False, debug=False, num_devices=NCORES
    )
    io = {}
    io["feat"] = nc.dram_tensor("feat", [P, D], F32, kind="ExternalInput")
    io["proj"] = nc.dram_tensor("proj", [P, PD], F32, kind="ExternalInput")
    io["ema"] = nc.dram_tensor("ema", [BP, PD], F32, kind="ExternalInput")
    io["cbk"] = nc.dram_tensor("cbk", [NC, D], F32, kind="ExternalInput")
    io["bank"] = nc.dram_tensor("bank", [NB, D], F32, kind="ExternalInput")
    io["bema"] = nc.dram_tensor("bema", [NB, PD], F32, kind="ExternalInput")
    io["selfcol"] = nc.dram_tensor("selfcol", [PT * 128], F32,
                                   kind="ExternalInput")
    io["out4"] = nc.dram_tensor("out4", [1, 4], F32, kind="ExternalOutput")
    io["ind_out"] = nc.dram_tensor("ind_out", [P], U32, kind="ExternalOutput")
    io["ncbk_dram"] = nc.dram_tensor("ncbk_dram", [NC, D], F32)
    io["cc_in"] = nc.dram_tensor("cc_in", [P], U32)
    io["idx_stage"] = nc.dram_tensor("idx_stage", [128, BP // 16], U32)
    io["cc_out"] = nc.dram_tensor("cc_out", [BP], U32, addr_space="Shared")

    with tile.TileContext(nc) as tc:
        with (
            tc.tile_pool(name="sb", bufs=1) as sb,
            tc.tile_pool(name="ld", bufs=3) as ld,
            tc.tile_pool(name="small", bufs=2) as small,
            tc.tile_pool(name="junk", bufs=2) as junk,
            tc.tile_pool(name="diag", bufs=4) as diag,
            tc.tile_pool(name="scr", bufs=3) as scr,
            tc.tile_pool(name="rhs", bufs=2) as rhspool,
            tc.tile_pool(name="part", bufs=1) as part,
            tc.tile_pool(name="psum", bufs=2, space="PSUM") as psum,
            tc.tile_pool(name="psT", bufs=2, space="PSUM") as psT,
        ):
            pools = {
                "sb": sb, "ld": ld, "small": small, "junk": junk,
                "diag": diag, "scr": scr, "rhs": rhspool, "part": part,
                "psum": psum, "psT": psT,
            }
            build_kernel_body(nc, tc, io, pools)
    nc.compile()
    return nc


_NC_CACHE = None


def _get_nc():
    global _NC_CACHE
    if _NC_CACHE is None:
        _NC_CACHE = build_nc()
    return _NC_CACHE


LAST_PARTS = None


def kernel(feat, proj_feat, proj_feat_ema, codebook, bank_vq_feat,
           bank_proj_feat_ema):
    global LAST_PARTS
    feat = np.ascontiguousarray(np.asarray(feat, dtype=np.float32))
    proj_feat = np.ascontiguousarray(np.asarray(proj_feat, dtype=np.float32))
    ema = np.ascontiguousarray(
        np.asarray(proj_feat_ema, dtype=np.float32).reshape(BP, PD)
    )
    codebook = np.ascontiguousarray(np.asarray(codebook, dtype=np.float32))
    bank = np.ascontiguousarray(np.asarray(bank_vq_feat, dtype=np.float32))
    bema = np.ascontiguousarray(
        np.asarray(bank_proj_feat_ema, dtype=np.float32)
    )

    nc = _get_nc()
    in_maps = []
    for c in range(NCORES):
        selfcol = np.full(PT * 128, -1.0, np.float32)
        selfcol[:P] = c * P + np.arange(P, dtype=np.float32)
        in_maps.append({
            "feat": feat[c],
            "proj": proj_feat[c],
            "ema": ema,
            "cbk": codebook,
            "bank": bank,
            "bema": bema,
            "selfcol": selfcol,
        })
    res = run_bass_kernel_spmd(nc, in_maps, core_ids=list(range(NCORES)))
    parts = np.stack([res.results[c]["out4"].reshape(4) for c in range(NCORES)])
    LAST_PARTS = {
        "parts": parts,
        "ind": np.stack([res.results[c]["ind_out"] for c in range(NCORES)]),
        "exec_time_ns": res.exec_time_ns,
    }
    with np.errstate(divide="ignore", invalid="ignore"):
        loss_batch = parts[:, 0] / parts[:, 1]
        loss_bank = parts[:, 2] / parts[:, 3]
        loss = (np.sum(loss_batch) + np.sum(loss_bank)) / (2.0 * B)
    return np.float32(loss)
